# revision 17
# baseline (speedup 1.0000x reference)
"""ArcFace (AngularPenaltySMLoss) over [32768, 8192] f32, distributed over
8 TRN2 NeuronCores, data-parallel on the batch dim.

The kernel is DMA-bound: every byte of cls_score must be read once
(128 MiB/core). The device does ONLY the memory-bound part: stream the
shard and compute per-row sums of exp(S*x) via ScalarE activation with
fused free-dim accumulation. The exp output itself is throwaway: it is
dumped to SBUF as saturating fp8e4 so a whole [128, 8192] tile fits one
ACTIVATE (8 KiB/partition; sub-f32 writes to PSUM are rejected by
neuronxcc — matmul/memset only). The accumulator taps pre-cast fp32 —
the loss is bit-identical to the f32-dump variant (rel err 1.6e-7
against the jax reference either way). The O(N) epilogue (target gather,
arcface numerator, log) runs on host in float64 off the [N] row
exp-sums — this removes the VectorE iota==label gather (which ran at
~97 G elem/s, nearly DMA rate), the GpSimd iota, the labels DMA, and
the on-device epilogue, all of which sat on the old 33 us tail.

Pipeline facts this layout is built around (measured via NTFF traces):
  - With enough x buffers the 16-SDMA stream runs gap-free at up to
    ~430 GB/s (SBUF-fabric-bound); on congested-HBM runs packets slow
    to ~340 GB/s, which no kernel structure can beat.
  - A DMA's completion SEMAPHORE lags its last data packet by ~15-35 us
    under load (write-receipt backlog). The x-buffer pool must cover
    lag + the ACT chain (~8.5 us/tile) + issue latency or the issue
    loop serializes at ~12.2+ us/tile: 4- and 5-buffer variants fell
    into that mode on real runs (417-434 us); 6 buffers never did.
  - Whole-tile DMAs maximize the 8-semaphore-lane reuse window
    (8 x 4 MiB in flight); finer-grained streams hit the lane-reuse
    guard earlier and serialize sooner in bad weather.
  - ScalarE consumes a tile in one 8192-col exp + accum-read = ~7.4 us
    (measured 7120 ns ACTIVATE, no SBUF-write throttle) < the ~9.7 us
    tile stream time, so ACT tracks the stream.

Layout per core: 32 row-tiles of [128, 8192] through a 6-deep pool on
the sync HWDGE ring. The last tile streams as two column-chunk DMAs
(4096 + 2560 cols) while its final 1536 cols are prefetched at kernel
start on the scalar ring and exp'd during ScalarE's otherwise-idle ramp
(first stream completion only lands ~20 us in) — that moves ~2.3 us of
exp out of the endgame and leaves only two short ACT chunks after the
final DMA completion. Row-sum partials go out as [128, 34] f32 in two
DMAs on the scalar ring: cols 0..30 (tiles 0..30) issued so their
completion hides under the endgame, cols 31..33 (tail chunks) at the
end (12 B/partition, ~2 us receipt on a drained queue).
"""

import numpy as np

from concourse import bacc, mybir, tile
from concourse.bass_utils import run_bass_kernel_spmd

N, C = 32768, 8192
N_CORES = 8
N_SHARD = N // N_CORES      # 4096 rows per core
P = 128                     # SBUF partitions
N_TILES = N_SHARD // P      # 32 tiles per core
S = 32.0
M = 0.5
EPS = 1e-7

_F32 = mybir.dt.float32
_F8 = mybir.dt.float8e4

# last tile: two streamed column chunks + one chunk prefetched at start
# (exp'd during ScalarE's early idle window, using the spare ~7.8 KiB of
# SBUF left by the 6 stream buffers)
TAIL_SPLITS = (0, 4096, 6656)        # streamed chunk starts; ends at PRE_COL
PRE_COL = 6656                       # cols [6656:8192) prefetched (6 KiB)
TAIL_COLS = len(TAIL_SPLITS)         # 2 streamed + 1 prefetched accum cols
MAIN_COLS = N_TILES - 1              # 31 accum cols for tiles 0..30
OUT_COLS = MAIN_COLS + TAIL_COLS     # 34


def build():
    nc = bacc.Bacc(None, target_bir_lowering=False)

    x_ext = nc.declare_dram_parameter("cls_score", [N_SHARD, C], _F32, isOutput=False)
    out_ext = nc.declare_dram_parameter("out", [P, OUT_COLS], _F32, isOutput=True)

    AF = mybir.ActivationFunctionType
    half = C // 2

    with tile.TileContext(nc) as tc:
        with (
            tc.tile_pool(name="xp", bufs=6) as xp,
            tc.tile_pool(name="ep", bufs=1) as ep,
            tc.tile_pool(name="tp", bufs=1) as tp,
            tc.tile_pool(name="st", bufs=1) as st,
        ):
            # separate main/tail accumulator tiles so the early out DMA
            # (reads main) has no WAR hazard against the tail accums
            sums_main = st.tile([P, MAIN_COLS], _F32)
            sums_tail = st.tile([P, TAIL_COLS], _F32)

            # prefetch the final columns of the last tile; their exp runs
            # in ScalarE's otherwise-idle ramp (first stream completion
            # arrives ~20 us in), shrinking the post-stream ACT chain
            pt = tp.tile([P, C - PRE_COL], _F32)
            nc.scalar.dma_start(
                out=pt[:], in_=x_ext[(N_TILES - 1) * P:, PRE_COL:])
            pe = ep.tile([P, C - PRE_COL], _F8)
            nc.scalar.activation(
                out=pe[:], in_=pt[:], func=AF.Exp, scale=S,
                accum_out=sums_tail[:, TAIL_COLS - 1:TAIL_COLS])

            for k in range(N_TILES - 1):
                xt = xp.tile([P, C], _F32)
                nc.sync.dma_start(out=xt[:], in_=x_ext[k * P:(k + 1) * P, :])
                et = ep.tile([P, C], _F8)
                nc.scalar.activation(
                    out=et[:], in_=xt[:], func=AF.Exp, scale=S,
                    accum_out=sums_main[:, k:k + 1])

            # main row-sum partials out; completes under the endgame
            nc.scalar.dma_start(out=out_ext[:, :MAIN_COLS], in_=sums_main[:])

            k = N_TILES - 1
            bounds = TAIL_SPLITS + (PRE_COL,)
            for j in range(len(TAIL_SPLITS) - 1):
                c0, c1 = bounds[j], bounds[j + 1]
                xt = xp.tile([P, c1 - c0], _F32)
                nc.sync.dma_start(
                    out=xt[:], in_=x_ext[k * P:(k + 1) * P, c0:c1])
                et = ep.tile([P, c1 - c0], _F8)
                nc.scalar.activation(
                    out=et[:], in_=xt[:], func=AF.Exp, scale=S,
                    accum_out=sums_tail[:, j:j + 1])

            nc.scalar.dma_start(out=out_ext[:, MAIN_COLS:], in_=sums_tail[:])

    nc.finalize()
    return nc


_NC_CACHE = {}


def _get_nc():
    if "nc" not in _NC_CACHE:
        _NC_CACHE["nc"] = build()
    return _NC_CACHE["nc"]


def make_in_maps(cls_score):
    cls_score = np.ascontiguousarray(np.asarray(cls_score, dtype=np.float32))
    return [
        {"cls_score": cls_score[i * N_SHARD:(i + 1) * N_SHARD]}
        for i in range(N_CORES)
    ]


def postprocess(results, cls_score, labels):
    """Host epilogue in float64 off the device per-row exp-sums."""
    cls_score = np.asarray(cls_score, dtype=np.float32)
    labels = np.asarray(labels).astype(np.int64)
    rowsum = np.empty((N,), dtype=np.float64)
    for i, r in enumerate(results):
        o = r["out"].astype(np.float64)                    # [P, OUT_COLS]
        main = o[:, :MAIN_COLS]
        tailv = o[:, MAIN_COLS:].sum(axis=1)               # [P]
        # shard row n = k*P + p  ->  main[p, k] (k < 31) or tailv[p]
        rowsum[i * N_SHARD:(i + 1) * N_SHARD] = np.concatenate(
            [main.T.reshape(-1), tailv])
    target = cls_score[np.arange(N), labels].astype(np.float64)
    t = np.clip(target, -1.0 + EPS, 1.0 - EPS)
    num = S * np.cos(np.arccos(t) + M)
    excl = rowsum - np.exp(S * target)
    L = num - np.log(np.exp(num) + excl)
    return np.float32(-np.mean(L))


def kernel(cls_score, labels):
    nc = _get_nc()
    in_maps = make_in_maps(cls_score)
    res = run_bass_kernel_spmd(nc, in_maps, core_ids=list(range(N_CORES)))
    return postprocess(res.results, cls_score, labels)


# revision 19
# speedup vs baseline: 1.0835x; 1.0835x over previous
"""ArcFace (AngularPenaltySMLoss) over [32768, 8192] f32, distributed over
8 TRN2 NeuronCores, data-parallel on the batch dim.

The kernel is DMA-bound: every byte of cls_score must be read once
(128 MiB/core). The device does ONLY the memory-bound part: stream the
shard and compute per-row sums of exp(S*x) via ScalarE activation with
fused free-dim accumulation. The exp output itself is throwaway: it is
dumped to SBUF as saturating fp8e4 so a whole [128, 8192] tile fits one
ACTIVATE (8 KiB/partition; sub-f32 writes to PSUM are rejected by
neuronxcc — matmul/memset only). The accumulator taps pre-cast fp32 —
the loss is bit-identical to the f32-dump variant (rel err 1.6e-7
against the jax reference either way). The O(N) epilogue (target gather,
arcface numerator, log) runs on host in float64 off the [N] row
exp-sums — this removes the VectorE iota==label gather (which ran at
~97 G elem/s, nearly DMA rate), the GpSimd iota, the labels DMA, and
the on-device epilogue, all of which sat on the old 33 us tail.

Pipeline facts this layout is built around (measured via NTFF traces):
  - With enough x buffers the 16-SDMA stream runs gap-free at up to
    ~430 GB/s (SBUF-fabric-bound); on congested-HBM runs packets slow
    to ~340 GB/s, which no kernel structure can beat.
  - A DMA's completion SEMAPHORE lags its last data packet by ~15-35 us
    under load (write-receipt backlog). The x-buffer pool must cover
    lag + the ACT chain (~8.5 us/tile) + issue latency or the issue
    loop serializes at ~12.2+ us/tile: 4- and 5-buffer variants fell
    into that mode on real runs (417-434 us); 6 buffers never did.
  - Whole-tile DMAs maximize the 8-semaphore-lane reuse window
    (8 x 4 MiB in flight); finer-grained streams hit the lane-reuse
    guard earlier and serialize sooner in bad weather.
  - ScalarE consumes a tile in one 8192-col exp + accum-read = ~7.4 us
    (measured 7120 ns ACTIVATE, no SBUF-write throttle) < the ~9.7 us
    tile stream time, so ACT tracks the stream.

Layout per core: 32 row-tiles of [128, 8192] through a 6-deep pool on
the sync HWDGE ring. The last tile streams as two column-chunk DMAs
(4096 + 2560 cols) while its final 1536 cols are prefetched at kernel
start on the scalar ring and exp'd during ScalarE's otherwise-idle ramp
(first stream completion only lands ~20 us in) — that moves ~2.3 us of
exp out of the endgame and leaves only two short ACT chunks after the
final DMA completion. Row-sum partials go out as [128, 34] f32 in two
DMAs on the scalar ring: cols 0..30 (tiles 0..30) issued so their
completion hides under the endgame, cols 31..33 (tail chunks) at the
end (12 B/partition, ~2 us receipt on a drained queue).
"""

import numpy as np

from concourse import bacc, mybir, tile
from concourse.bass_utils import run_bass_kernel_spmd

N, C = 32768, 8192
N_CORES = 8
N_SHARD = N // N_CORES      # 4096 rows per core
P = 128                     # SBUF partitions
N_TILES = N_SHARD // P      # 32 tiles per core
S = 32.0
M = 0.5
EPS = 1e-7

_F32 = mybir.dt.float32
_F8 = mybir.dt.float8e4

# last tile: two streamed column chunks + one chunk prefetched at start
# (exp'd during ScalarE's early idle window, using the spare ~7.8 KiB of
# SBUF left by the 6 stream buffers)
TAIL_SPLITS = (0, 4096, 6656)        # streamed chunk starts; ends at PRE_COL
PRE_COL = 6656                       # cols [6656:8192) prefetched (6 KiB)
TAIL_COLS = len(TAIL_SPLITS)         # 2 streamed + 1 prefetched accum cols
MAIN_COLS = N_TILES - 2              # 30 accum cols for tiles 0..29
T30_COLS = 2                         # tile 30 streams as two 4096-col halves
OUT_COLS = MAIN_COLS + T30_COLS + TAIL_COLS  # 35


def build():
    nc = bacc.Bacc(None, target_bir_lowering=False)

    x_ext = nc.declare_dram_parameter("cls_score", [N_SHARD, C], _F32, isOutput=False)
    out_ext = nc.declare_dram_parameter("out", [P, OUT_COLS], _F32, isOutput=True)

    AF = mybir.ActivationFunctionType
    half = C // 2

    with tile.TileContext(nc) as tc:
        with (
            tc.tile_pool(name="xp", bufs=6) as xp,
            tc.tile_pool(name="ep", bufs=1) as ep,
            tc.tile_pool(name="tp", bufs=1) as tp,
            tc.tile_pool(name="st", bufs=1) as st,
        ):
            # separate main/tail accumulator tiles so the early out DMA
            # (reads main) has no WAR hazard against the tail accums
            sums_main = st.tile([P, MAIN_COLS], _F32)
            sums_t30 = st.tile([P, T30_COLS], _F32)
            sums_tail = st.tile([P, TAIL_COLS], _F32)

            # prefetch the final columns of the last tile; their exp runs
            # in ScalarE's otherwise-idle ramp (first stream completion
            # arrives ~20 us in), shrinking the post-stream ACT chain
            pt = tp.tile([P, C - PRE_COL], _F32)
            nc.scalar.dma_start(
                out=pt[:], in_=x_ext[(N_TILES - 1) * P:, PRE_COL:])
            pe = ep.tile([P, C - PRE_COL], _F8)
            nc.scalar.activation(
                out=pe[:], in_=pt[:], func=AF.Exp, scale=S,
                accum_out=sums_tail[:, TAIL_COLS - 1:TAIL_COLS])

            for k in range(N_TILES - 2):
                xt = xp.tile([P, C], _F32)
                nc.sync.dma_start(out=xt[:], in_=x_ext[k * P:(k + 1) * P, :])
                et = ep.tile([P, C], _F8)
                nc.scalar.activation(
                    out=et[:], in_=xt[:], func=AF.Exp, scale=S,
                    accum_out=sums_main[:, k:k + 1])

            # tile 30 as two half-DMAs: its first half completes before
            # tile 29's exp chain ends, so ScalarE never idles waiting on
            # the whole-tile completion right before the endgame
            k30 = N_TILES - 2
            for h in range(T30_COLS):
                c0 = h * (C // T30_COLS)
                c1 = c0 + C // T30_COLS
                xt = xp.tile([P, c1 - c0], _F32)
                nc.sync.dma_start(
                    out=xt[:], in_=x_ext[k30 * P:(k30 + 1) * P, c0:c1])
                et = ep.tile([P, c1 - c0], _F8)
                nc.scalar.activation(
                    out=et[:], in_=xt[:], func=AF.Exp, scale=S,
                    accum_out=sums_t30[:, h:h + 1])

            # rows 0..30 partials out; completes under the endgame
            nc.scalar.dma_start(out=out_ext[:, :MAIN_COLS], in_=sums_main[:])
            nc.scalar.dma_start(
                out=out_ext[:, MAIN_COLS:MAIN_COLS + T30_COLS], in_=sums_t30[:])

            k = N_TILES - 1
            bounds = TAIL_SPLITS + (PRE_COL,)
            for j in range(len(TAIL_SPLITS) - 1):
                c0, c1 = bounds[j], bounds[j + 1]
                xt = xp.tile([P, c1 - c0], _F32)
                nc.sync.dma_start(
                    out=xt[:], in_=x_ext[k * P:(k + 1) * P, c0:c1])
                et = ep.tile([P, c1 - c0], _F8)
                nc.scalar.activation(
                    out=et[:], in_=xt[:], func=AF.Exp, scale=S,
                    accum_out=sums_tail[:, j:j + 1])

            nc.scalar.dma_start(
                out=out_ext[:, MAIN_COLS + T30_COLS:], in_=sums_tail[:])

    nc.finalize()
    return nc


_NC_CACHE = {}


def _get_nc():
    if "nc" not in _NC_CACHE:
        _NC_CACHE["nc"] = build()
    return _NC_CACHE["nc"]


def make_in_maps(cls_score):
    cls_score = np.ascontiguousarray(np.asarray(cls_score, dtype=np.float32))
    return [
        {"cls_score": cls_score[i * N_SHARD:(i + 1) * N_SHARD]}
        for i in range(N_CORES)
    ]


def postprocess(results, cls_score, labels):
    """Host epilogue in float64 off the device per-row exp-sums."""
    cls_score = np.asarray(cls_score, dtype=np.float32)
    labels = np.asarray(labels).astype(np.int64)
    rowsum = np.empty((N,), dtype=np.float64)
    for i, r in enumerate(results):
        o = r["out"].astype(np.float64)                    # [P, OUT_COLS]
        main = o[:, :MAIN_COLS]                            # [P, 30]
        t30v = o[:, MAIN_COLS:MAIN_COLS + T30_COLS].sum(axis=1)
        tailv = o[:, MAIN_COLS + T30_COLS:].sum(axis=1)    # [P]
        # shard row n = k*P + p -> main[p, k] (k<30), t30v[p], tailv[p]
        rowsum[i * N_SHARD:(i + 1) * N_SHARD] = np.concatenate(
            [main.T.reshape(-1), t30v, tailv])
    target = cls_score[np.arange(N), labels].astype(np.float64)
    t = np.clip(target, -1.0 + EPS, 1.0 - EPS)
    num = S * np.cos(np.arccos(t) + M)
    excl = rowsum - np.exp(S * target)
    L = num - np.log(np.exp(num) + excl)
    return np.float32(-np.mean(L))


def kernel(cls_score, labels):
    nc = _get_nc()
    in_maps = make_in_maps(cls_score)
    res = run_bass_kernel_spmd(nc, in_maps, core_ids=list(range(N_CORES)))
    return postprocess(res.results, cls_score, labels)
